# revision 56
# baseline (speedup 1.0000x reference)
"""Self-contained Trainium2 Bass kernel for nn_Denoiser_77841987273333.

kernel(**inputs) takes the FULL inputs (as produced by setup_inputs) and returns
the FULL [4, 8192, 3] output. Internally: shards batch*half across 8 NeuronCores
(core = 2*b + half; each core handles 4096 query rows of one batch with the full
8192-point candidate set), compiles one SPMD Bass program, runs it on cores 0-7
via concourse.bass_utils.run_bass_kernel_spmd, and reassembles the output.

Algorithm per core (all on-device):
  phi = relu(W1 x + b1) (64-d per point); z = x_i.x_j - 0.5|x_j|^2 (PE,
  written fp16); exact top-16 per row via DVE max8/max_index/match_replace
  (self excluded via a -60000 diagonal); one GPSIMD gather from a single
  [128, 8192] table fetches phi (rows 0:64) + coords (rows 64:67) for all 16
  neighbors; pair conv h2 = lrelu(Wc2 lrelu(A x_i + B x_j + bc1) + bc2) with
  A = Wc1a+Wc1c, B = Wc1b-Wc1c; attention logits s = u~.phi_j + v.h2_ij
  where u~ = Gu phi_i + Hu h2s_i + gu and v = Gv phi_i + Hv h2s_i + gv are
  fully host-folded from (Wq, Wk, W2, Wc3) and precomputed for every query
  point at setup — the q/k projections never run on device; exp on ACT with
  a fused sum accumulator (scores are O(0.05), no max-subtraction needed);
  the weighted coordinate sums and the exp-sum denominators ship separately
  and the host divides.

The tile loop is software-pipelined across 3 tile-generations so that no DVE
instruction ever waits on the score/weight DMA round-trips: the DVE top-k
scans (5 passes over [128, 8192] per 128-row tile) run back-to-back as the
critical path, with gathers + elementwise on Pool, conv/score matmuls on PE,
and PSUM evacuation + exp on ACT overlapped underneath. Cost-model estimate
~1.66 ms/core vs 3.28 ms for the previous version of this kernel.
"""
from contextlib import ExitStack

import numpy as np

import concourse.bass as bass
import concourse.mybir as mybir
import concourse.tile as tile
from concourse.bass_utils import run_bass_kernel_spmd
from concourse.masks import make_identity

F32 = mybir.dt.float32
F32R = mybir.dt.float32r
F16 = mybir.dt.float16
U16 = mybir.dt.uint16
AF = mybir.ActivationFunctionType
LRELU = 0.01
NEG = -60000.0
NEGBIG = -60000.0

B, N, NQ, K = 4, 8192, 4096, 16
N_CORES = 8
NH = N // 2
GP = 128          # gather partitions: 64 phi + 3 xyz + pad (full 128 so all
                  # 8 GPSIMD cores read valid per-group index streams)

# (name, [p, f], dtype) — fp16 for operands that pair with fp16 gathered data
WSPECS = [
    ("W1T", [3, 64], F32), ("Wc1A", [3, 64], F32), ("Wc1B", [3, 64], F32),
    ("Wc1S", [3, 64], F32), ("Wc2T", [64, 64], F16),
    ("GuT", [64, 64], F32), ("HuT", [64, 64], F32),
    ("GvT", [64, 64], F32), ("HvT", [64, 64], F32),
    ("b1c", [64, 1], F32), ("bc1c", [64, 1], F32), ("bc2c", [64, 1], F32),
    ("guc", [64, 1], F32), ("gvc", [64, 1], F32),
]


def build(nc: bass.Bass, n=N, nq=NQ):
    n_tiles = nq // 128
    GC = 128 * K

    xTs_d = nc.dram_tensor("xTs", [4, n], F32, kind="ExternalInput")
    lhq_d = nc.dram_tensor("lhq", [4, nq], F32, kind="ExternalInput")
    wd = {m: nc.dram_tensor(m, s, d, kind="ExternalInput")
          for m, s, d in WSPECS}
    out_d = nc.dram_tensor("out", [3, nq], F32, kind="ExternalOutput")
    den_d = nc.dram_tensor("den", [128, nq // 128], F32, kind="ExternalOutput")
    idx_dramW = nc.dram_tensor("idxw_scratch", [16, 128], U16, kind="Internal")
    idx_dramW8 = nc.dram_tensor("idxw8_scratch", [128, 128], U16, kind="Internal")
    s_dram = nc.dram_tensor("s_scratch", [1, GC], F32, kind="Internal")
    w_dram = nc.dram_tensor("w_scratch", [128, K], F32, kind="Internal")

    r32 = lambda ap: ap  # fp32r rejected by BIR verifier unless producer rounds

    with tile.TileContext(nc) as tc, ExitStack() as ctx:
        const = ctx.enter_context(tc.tile_pool(name="const", bufs=1))
        xTs = const.tile([4, n], F32)
        lhq = const.tile([4, nq], F32)
        # single fp32 gather table: phi at rows 0:64, coords at 64:67
        # (GPSIMD indirect copy needs >=4-byte elements per index)
        G = const.tile([GP, n], F32)
        # Wc1B multiplies gathered coords that live at partitions 64:67, and
        # PE requires lhsT/rhs to share a base partition — home it at 64.
        W = {m: const.tile([67, s[1]] if m == "Wc1B" else s, d, name=m,
                           tag=m)
             for m, s, d in WSPECS}
        nbig_id = const.tile([128, 128], F32)
        ones16 = const.tile([64, 1], F16)
        ones32 = const.tile([64, 1], F32)

        # lhq + xTs first in the DMA queue: the tile-0 z matmuls need them
        # before anything else.
        nc.sync.dma_start(out=lhq[:, 0:256], in_=lhq_d[:, 0:256])
        for xc in range(8):
            xsl = bass.ts(xc, n // 8)
            nc.sync.dma_start(out=xTs[:, xsl], in_=xTs_d[:, xsl])
        nc.sync.dma_start(out=lhq[:, 256:nq], in_=lhq_d[:, 256:nq])
        for m, s, d in WSPECS:
            tgt = W[m][64:67, :] if m == "Wc1B" else W[m][:]
            nc.sync.dma_start(out=tgt, in_=wd[m][:])
        nc.gpsimd.memset(G[64:128, :], 0.0)
        nc.sync.dma_start(out=G[64:67, :], in_=xTs_d[0:3, :])
        make_identity(nc, nbig_id[:])
        nc.scalar.mul(out=nbig_id[:], in_=nbig_id[:], mul=NEGBIG)
        nc.vector.memset(ones16[:], 1.0)
        nc.vector.memset(ones32[:], 1.0)

        # Setup compute, per 512-point chunk:
        #   phi = relu(W1 x + b1) into G rows 0:64
        #   h2s = self-pair conv (depends only on x)
        #   u~ = Gu phi + Hu h2s + gu, v = Gv phi + Hv h2s + gv for ALL
        #   query points (both depend only on constants, so hoisting them
        #   out of the tile loop removes the conv->u~->score serial chain)
        uta = const.tile([64, nq], F16)
        vta = const.tile([64, nq], F16)
        zpool = ctx.enter_context(tc.tile_pool(name="z", bufs=3))
        pools = ctx.enter_context(tc.tile_pool(name="work", bufs=2))
        spool = ctx.enter_context(tc.tile_pool(name="small", bufs=2))
        zps = ctx.enter_context(tc.tile_pool(name="zps", bufs=4, space="PSUM"))
        cps = ctx.enter_context(tc.tile_pool(name="cps", bufs=2, space="PSUM"))

        T = {}

        def stageCz(t):
            trows = bass.ts(t, 128)
            z = zpool.tile([128, n], F16, tag="z")
            for ch in range(n // 512):
                zp = zps.tile([128, 512], F32, tag="zp")
                nc.tensor.matmul(zp[:], r32(lhq[:, trows]),
                                 r32(xTs[0:4, bass.ts(ch, 512)]),
                                 start=True, stop=True)
                nc.scalar.activation(z[:, bass.ts(ch, 512)], zp[:], AF.Copy)
            nc.vector.tensor_add(z[:, trows], z[:, trows], nbig_id[:])

            m1 = spool.tile([128, 8], F16, tag="m1")
            m2 = spool.tile([128, 8], F16, tag="m2")
            idx = spool.tile([128, K], U16, tag="idx")
            nc.vector.max(out=m1[:], in_=z[:])
            nc.vector.max_index(out=idx[:, 0:8], in_max=m1[:], in_values=z[:])
            nc.vector.match_replace(out=z[:], in_to_replace=m1[:],
                                    in_values=z[:], imm_value=NEG)
            nc.vector.max(out=m2[:], in_=z[:])
            nc.vector.max_index(out=idx[:, 8:16], in_max=m2[:], in_values=z[:])

            nc.sync.dma_start(out=idx_dramW.rearrange("p s -> s p"), in_=idx[:])
            nc.sync.dma_start(
                out=idx_dramW8[:].rearrange("(g p) s -> g p s", g=8),
                in_=idx_dramW[None].broadcast_to([8, 16, 128]))
            idxw = spool.tile([GP, 128], U16, tag="idxw")
            nc.sync.dma_start(out=idxw[:], in_=idx_dramW8[:])
            T[t] = dict(idxw=idxw)

        # Tiles 0 and 1 depend only on xTs/lhq — author their z builds and
        # top-16 scans BEFORE the setup ladder so PE does z(0)/z(1) first and
        # the DVE scans overlap the phi/u~/v setup compute.
        stageCz(0)
        stageCz(1)

        with tc.tile_pool(name="setup_sb", bufs=2) as fsb:
            for c in range(n // 512):
                sl = bass.ts(c, 512)
                pp = cps.tile([128, 512], F32, tag="cp1")
                nc.tensor.matmul(pp[0:64, :], r32(W["W1T"][:]),
                                 r32(xTs[0:3, sl]), start=True, stop=True)
                nc.scalar.activation(G[0:64, sl], pp[0:64, :], AF.Relu,
                                     bias=W["b1c"][:])
                if c >= nq // 512:
                    continue
                nc.tensor.matmul(pp[64:128, :], r32(W["Wc1S"][:]),
                                 r32(xTs[0:3, sl]), start=True, stop=True)
                h1s = fsb.tile([64, 512], F16, tag="h1s")
                nc.scalar.activation(h1s[:], pp[64:128, :], AF.Lrelu,
                                     bias=W["bc1c"][:], alpha=LRELU)
                p2 = cps.tile([128, 512], F32, tag="cp2")
                nc.tensor.matmul(p2[0:64, :], W["Wc2T"][:], h1s[:],
                                 start=True, stop=True)
                h2s = fsb.tile([64, 512], F32, tag="h2s")
                nc.scalar.activation(h2s[:], p2[0:64, :], AF.Lrelu,
                                     bias=W["bc2c"][:], alpha=LRELU)
                pu = cps.tile([128, 512], F32, tag="cp1")
                for hh, (Gm, Hm, bm, tgt) in enumerate(
                        (("GuT", "HuT", "guc", uta), ("GvT", "HvT", "gvc", vta))):
                    po = pu[bass.ts(hh, 64), :]
                    nc.tensor.matmul(po, W[Gm][:], G[0:64, sl],
                                     start=True, stop=False)
                    nc.tensor.matmul(po, W[Hm][:], h2s[:],
                                     start=False, stop=True)
                    nc.scalar.activation(tgt[:, sl], po, AF.Identity,
                                         bias=W[bm][:])

        # -------------------------------------------------------------------
        # Software-pipelined tile loop. Iteration `it` authors, in order:
        #   A(it-1): pair conv + self conv + u~/v + score muls + score
        #            matmuls + s DMA round-trip        (PE/ACT/Pool, no DVE)
        #   B(it-2): weighted-coord product wx[64:67] = kA[64:67] * w
        #            (Pool; last reader of kA(it-2), authored before the
        #            kA(it) realloc in C so bufs=2 suffices)
        #   C(it):   z matmul/copy + top-16 scans + idx prep + gathers
        #   D(it-1): softmax (DVE smalls ready since mid-scan) + w DMA into
        #            wx rows 64:67
        #   E(it-2): per-row weighted sum reduce + output DMA
        # This keeps the DVE queue free of any op that would wait on the
        # s/w DMA round-trips: every DVE op is data-ready when reached.
        # -------------------------------------------------------------------

        def stageCg(t):
            # gather: phi (rows 0:64) + xyz (64:67), in 1024-column chunks
            # (ISA limit on indirect-copy dst count)
            idxw = T[t]["idxw"]
            kA = pools.tile([GP, GC], F32, tag="kA")
            for gc in range(GC // 1024):
                nc.gpsimd.indirect_copy(kA[:, bass.ts(gc, 1024)], G[:],
                                        idxw[:, bass.ts(gc, 64)], True)
            T[t]["kA"] = kA

        def stageA(t):
            trows = bass.ts(t, 128)
            kA = T[t]["kA"]
            h2t = pools.tile([64, GC], F16, tag="h2t")
            T[t]["h2t"] = h2t
            # pair conv: h1 = lrelu(A x_i + B x_j + bc1); h2 -> h2t
            for cc in range(GC // 512):
                sl = bass.ts(cc, 512)
                p1 = cps.tile([128, 512], F32, tag="cp1")
                rep_c = xTs[0:3, bass.ds(t * 128 + cc * 32, 32)].to_broadcast(
                    [3, 32, K])
                nc.tensor.matmul(p1[0:64, :], r32(W["Wc1A"][:]), r32(rep_c),
                                 start=True, stop=False)
                nc.tensor.matmul(p1[0:64, :], W["Wc1B"][64:67, :],
                                 kA[64:67, sl], start=False, stop=True)
                h1 = spool.tile([64, 512], F16, tag="h1")
                nc.scalar.activation(h1[:], p1[0:64, :], AF.Lrelu,
                                     bias=W["bc1c"][:], alpha=LRELU)
                p2 = cps.tile([128, 512], F32, tag="cp2")
                nc.tensor.matmul(p2[0:64, :], W["Wc2T"][:], h1[:],
                                 start=True, stop=True)
                nc.scalar.activation(h2t[:, sl], p2[0:64, :], AF.Lrelu,
                                     bias=W["bc2c"][:], alpha=LRELU)

            # scores s = u~.phi_j + v.h2_ij  (elementwise on Pool, sum on PE)
            kfv = kA[0:64, :].rearrange("c (r j) -> c r j", j=K)
            nc.gpsimd.tensor_mul(kfv, kfv,
                                 uta[:, trows].to_broadcast([64, 128, K]))
            h2v = h2t[:].rearrange("c (r j) -> c r j", j=K)
            nc.gpsimd.tensor_mul(h2v, h2v,
                                 vta[:, trows].to_broadcast([64, 128, K]))
            # wx rows 0:1 stage the score row; rows 64:67 later hold the
            # broadcast softmax weights then the weighted coords (disjoint
            # partitions, ordered by the s -> softmax -> w data chain).
            wx = pools.tile([67, GC], F32, tag="wx")
            for cc in range(GC // 512):
                sl = bass.ts(cc, 512)
                sp = cps.tile([128, 512], F32, tag="cp2")
                nc.tensor.matmul(sp[0:1, :], ones32[:], kA[0:64, sl],
                                 start=True, stop=False)
                nc.tensor.matmul(sp[0:1, :], ones16[:], h2t[:, sl],
                                 start=False, stop=True)
                nc.scalar.activation(wx[0:1, sl], sp[0:1, :], AF.Copy)
            nc.sync.dma_start(out=s_dram[:], in_=wx[0:1, :])
            st = spool.tile([128, 2 * K + 8], F32, tag="st")
            nc.sync.dma_start(out=st[:, 0:K],
                              in_=s_dram.rearrange("o (r j) -> (o r) j", j=K))
            T[t]["wx"], T[t]["st"] = wx, st

        def stageD(t):
            # Unnormalized softmax, ACT only (scores are O(0.05) so exp needs
            # no max-subtraction). The accumulated denominator ships to the
            # host, which divides — no DVE op ever waits on the score DMA.
            st, wx = T[t]["st"], T[t]["wx"]
            nc.scalar.activation(st[:, K:2 * K], st[:, 0:K], AF.Exp,
                                 accum_out=st[:, 2 * K:2 * K + 1])
            nc.sync.dma_start(out=w_dram[:], in_=st[:, K:2 * K])
            nc.sync.dma_start(
                out=wx[64:67, :],
                in_=w_dram.rearrange("r j -> (r j)")[None, :].broadcast_to(
                    [3, GC]))
            nc.sync.dma_start(out=den_d[:, t:t + 1],
                              in_=st[:, 2 * K:2 * K + 1])

        def stageE(t):
            # weighted coords + per-row sum, both on DVE: their inputs (kA
            # and the DMA'd weights in wx) are ready a full window ahead, so
            # neither op can stall the in-order DVE stream.
            trows = bass.ts(t, 128)
            kA, wx = T[t]["kA"], T[t]["wx"]
            nc.vector.tensor_mul(wx[64:67, :], kA[64:67, :], wx[64:67, :])
            nx = spool.tile([67, 128], F32, tag="nx")
            nc.vector.tensor_reduce(
                nx[64:67, :],
                wx[64:67, :].rearrange("c (r j) -> c r j", j=K),
                axis=mybir.AxisListType.X, op=mybir.AluOpType.add)
            nc.sync.dma_start(out=out_d[:, trows], in_=nx[64:67, :])
            del T[t]

        for it in range(n_tiles + 2):
            if 1 <= it <= n_tiles:
                stageA(it - 1)
            if it < n_tiles:
                if it >= 2:
                    stageCz(it)
                stageCg(it)
            if 1 <= it <= n_tiles:
                stageD(it - 1)
            if 2 <= it:
                stageE(it - 2)
    return nc


def prep_weights(w: dict):
    f32 = lambda a: np.ascontiguousarray(a).astype(np.float32)
    W1, b1, W2, b2 = w["W1"], w["b1"], w["W2"], w["b2"]
    Wc1, bc1, Wc2, bc2 = w["Wc1"], w["bc1"], w["Wc2"], w["bc2"]
    Wc3, bc3, Wq, bq, Wk, bk = w["Wc3"], w["bc3"], w["Wq"], w["bq"], w["Wk"], w["bk"]
    M = Wk.T @ Wq
    c = Wk.T @ bq
    M00, M01 = M[:128, :128], M[:128, 128:]
    M10, M11 = M[128:, :128], M[128:, 128:]
    Gu = W2.T @ M00 @ W2
    Hu = W2.T @ M01 @ Wc3
    gu = W2.T @ (M00 @ b2 + M01 @ bc3 + c[:128])
    Gv = Wc3.T @ M10 @ W2
    Hv = Wc3.T @ M11 @ Wc3
    gv = Wc3.T @ (M10 @ b2 + M11 @ bc3 + c[128:])
    A_ = Wc1[:, 0:3] + Wc1[:, 6:9]
    B_ = Wc1[:, 3:6] - Wc1[:, 6:9]
    S_ = Wc1[:, 0:3] + Wc1[:, 3:6]
    f16 = lambda a: np.ascontiguousarray(a).astype(np.float16)
    return {
        "W1T": f32(W1.T), "Wc1A": f32(A_.T), "Wc1B": f32(B_.T),
        "Wc1S": f32(S_.T), "Wc2T": f16(Wc2.T),
        "GuT": f32(Gu.T), "HuT": f32(Hu.T), "GvT": f32(Gv.T), "HvT": f32(Hv.T),
        "b1c": f32(b1[:, None]), "bc1c": f32(bc1[:, None]),
        "bc2c": f32(bc2[:, None]),
        "guc": f32(gu[:, None]), "gvc": f32(gv[:, None]),
    }


def prep_xts(x_b: np.ndarray, r0: int, n=N, nq=NQ):
    xr = np.roll(np.asarray(x_b, np.float32), -r0, axis=0)
    xTs = np.zeros((4, n), np.float32)
    xTs[0:3] = xr.T
    xTs[3] = -0.5 * (xr * xr).sum(-1)
    lhq = np.ones((4, nq), np.float32)
    lhq[0:3] = xr.T[:, 0:nq]
    return xTs, lhq


# ---------------------------------------------------------------------------
# Sync legalizer: the walrus in this container encodes at most ~2 sync
# commands per instruction; Tile emits up to 12 inline waits. Split excess
# waits into standalone EventSemaphore instructions (same engine, directly
# before the instruction) — semantically identical (engine blocks on each
# wait in order before issuing).
# ---------------------------------------------------------------------------
import json as _json

import concourse.bass2jax as _bass2jax
import concourse.bass_utils as _bass_utils


def _legalize_sync(bir_json):
    d = _json.loads(bir_json)
    for fn in d["functions"]:
        for bb in fn["blocks"]:
            out = []
            for inst in bb["instructions"]:
                si = inst.get("sync_info")
                waits = (si or {}).get("on_wait") or []
                budget = 1  # keep at most one inline wait per instruction
                if len(waits) > budget:
                    split, keep = waits[:-budget], waits[-budget:]
                    for i, w in enumerate(split):
                        out.append({
                            "debug": inst.get("debug", 0),
                            "engine": inst["engine"],
                            "ins": [], "outs": [],
                            "name": f"{inst['name']}-sw{i}",
                            "opcode": "EventSemaphore",
                            "sync_info": {"on_update": [], "on_wait": [w]},
                        })
                    si["on_wait"] = keep
                out.append(inst)
            bb["instructions"] = out
    return _json.dumps(d).encode()


_orig_compile_bir_kernel = _bass_utils.compile_bir_kernel


def _patched_compile_bir_kernel(bir_json, tmpdir, neff_name="file.neff"):
    return _orig_compile_bir_kernel(_legalize_sync(bir_json), tmpdir,
                                    neff_name=neff_name)


if _bass_utils.compile_bir_kernel is not _patched_compile_bir_kernel:
    _bass_utils.compile_bir_kernel = _patched_compile_bir_kernel
    _bass2jax.compile_bir_kernel = _patched_compile_bir_kernel


_CACHE = {}


def _get_nc():
    if "nc" not in _CACHE:
        nc = bass.Bass("TRN2")
        build(nc)
        _CACHE["nc"] = nc
    return _CACHE["nc"]


def kernel(x, global_feat, W1, b1, W2, b2, Wc1, bc1, Wc2, bc2, Wc3, bc3,
           Wq, bq, Wk, bk, _profile=None):
    del global_feat  # unused by the reference forward
    x = np.asarray(x, np.float32)
    w = prep_weights(dict(W1=W1, b1=b1, W2=W2, b2=b2, Wc1=Wc1, bc1=bc1,
                          Wc2=Wc2, bc2=bc2, Wc3=Wc3, bc3=bc3, Wq=Wq, bq=bq,
                          Wk=Wk, bk=bk))
    in_maps = []
    for core in range(N_CORES):
        b, half = core // 2, core % 2
        m = dict(w)
        m["xTs"], m["lhq"] = prep_xts(x[b], half * NQ)
        in_maps.append(m)

    nc = _get_nc()
    kwargs = dict(_profile) if _profile else {}
    res = run_bass_kernel_spmd(nc, in_maps, core_ids=list(range(N_CORES)),
                               **kwargs)
    out = np.zeros((B, N, 3), np.float32)
    for core in range(N_CORES):
        b, half = core // 2, core % 2
        den = res.results[core]["den"].T.reshape(NQ)       # [tile, row] -> row
        out[b, half * NQ:(half + 1) * NQ] = \
            res.results[core]["out"].T / den[:, None]
    if _profile is not None and isinstance(_profile, dict):
        _profile["exec_time_ns"] = res.exec_time_ns
        _profile["res"] = res
    return out
